# revision 19
# baseline (speedup 1.0000x reference)
"""Trainium2 Bass kernel for nn_ConstrainedEnhancementModel.

Contract: kernel(**inputs) takes the FULL unsharded inputs (as produced by
reference.setup_inputs()) and returns the FULL [4096, 2000, 6] float32 output.

Strategy (pure data parallel over 8 NeuronCores, 512 batch rows each):
  - Feature-major MLP chain: every hidden activation is stored [feat, batch]
    so torch-layout weights [fan_in, fan_out] are directly the matmul lhsT.
  - The final layer flips to batch-major: lhsT = h5 (feature-major) slices,
    rhs = W6 tiles, so output DMA writes are contiguous.
  - The constraint/interpolation epilogue is folded into the final matmul:
        out = h5 @ (W6 * c_dec) + x @ G + ones * (b6 * c_dec)
    where G is a sparse constant [600, 12000] matrix holding the linear
    interpolation + anchor/blend coefficients.  G contributions are exact
    f32 (anchor timesteps reproduce the input bit-exactly); the decoded
    path is bf16 (it only ever enters scaled by 0.2 or in the tail).
  - DMA instruction count is minimized (each DMA_DIRECT2D costs ~600ns of
    serialized descriptor-gen on its issuing engine queue): weights/biases
    are host-packed into a handful of wide tensors, W6 streams in 7 block
    DMAs, outputs leave in 28 block DMAs of [128, 1920] f32.
  - x is host-transposed into the window-blocked xre layout (pure layout
    transform), so no on-device transposes are needed.
"""

import numpy as np
import ml_dtypes

import bass_rust
import concourse.bass as bass
import concourse.bacc as bacc
import concourse.mybir as mybir
import concourse.tile as tile
from concourse import bass_utils

F32 = mybir.dt.float32
BF16 = mybir.dt.bfloat16
BF16_NP = ml_dtypes.bfloat16
F8 = mybir.dt.float8e4
F8_NP = ml_dtypes.float8_e4m3

# Problem config (hardcoded; must match the reference)
LOW_T = 100
HIGH_T = 2000
FEAT = 6
HID = 256
NUM_CLASSES = 10
LBL_DIM = 16
UP = 20
B = 4096
NCORES = 8
BC = B // NCORES          # 512 batch rows per core
NBT = BC // 128           # 4 batch tiles per core
D_IN = LOW_T * FEAT       # 600
D_OUT = HIGH_T * FEAT     # 12000
NW = 25                   # output windows (80 timesteps * 6 feats = 480 cols)
WT = 480
NI4 = 7                   # ceil(25/4) groups of 4 windows

# packed encoder-weight column offsets in wenc [128, 6400] fp8
OFF_W1 = 0          # 7 blocks of 512
OFF_W2 = 3584       # 4 k-tiles of 256
OFF_W3 = 4608       # 2 k-tiles of 128
OFF_W4A = 4864      # 256
OFF_W5 = 5120       # 2 k-tiles of 512
OFF_W4B = 6144      # 256 (rows 0-15 real, rest zero; plane-paired with W4A)
WENC_COLS = 6400
# packed bias column offsets in bias [128, 13] f32
OFF_B1, OFF_B2, OFF_B3, OFF_B4, OFF_B5 = 0, 4, 6, 7, 9


def _build_nc():
    """Build the single-core Bass program (SPMD: same program on all 8)."""
    nc = bacc.Bacc("TRN2", target_bir_lowering=False, debug=False)

    xre_d = nc.dram_tensor("xre", [128, NI4 * 512], BF16, kind="ExternalInput")
    xre8_d = nc.dram_tensor("xre8", [128, NI4 * 512], F8, kind="ExternalInput")
    lab_d = nc.dram_tensor("labf", [1, BC], BF16, kind="ExternalInput")
    wenc_d = nc.dram_tensor("wenc", [128, WENC_COLS], F8, kind="ExternalInput")
    bias_d = nc.dram_tensor("bias", [128, 13], F32, kind="ExternalInput")
    emb_d = nc.dram_tensor("embT", [NUM_CLASSES, LBL_DIM], BF16, kind="ExternalInput")
    iota_d = nc.dram_tensor("iota10", [NUM_CLASSES, 1], F32, kind="ExternalInput")
    g_d = nc.dram_tensor("gmat", [128, NI4 * WT], BF16, kind="ExternalInput")
    w6_d = nc.dram_tensor("w6r", [128, NW * 4 * WT], F8, kind="ExternalInput")
    y_d = nc.dram_tensor("y", [BC, D_OUT], BF16, kind="ExternalOutput")

    RELU = mybir.ActivationFunctionType.Relu

    with tile.TileContext(nc) as tc:
        with (
            tc.tile_pool(name="const", bufs=1) as cp,
            tc.tile_pool(name="w6pool", bufs=2) as wp,
            tc.tile_pool(name="outpool", bufs=4) as op,
            tc.tile_pool(name="ppool", bufs=8, space="PSUM") as pm,
        ):
            # ---- persistent SBUF tensors ----
            xre_b = cp.tile([128, NI4 * 512], BF16, tag="xre_b", name="xre_b")
            cxre8 = cp.tile([128, NI4 * 512], F8, tag="cxre8", name="cxre8")
            cwenc = cp.tile([128, WENC_COLS], F8, tag="cwenc", name="cwenc")
            cbias = cp.tile([128, 13], F32, tag="cbias", name="cbias")
            cemb = cp.tile([NUM_CLASSES, LBL_DIM], BF16, tag="cemb", name="cemb")
            ciota = cp.tile([NUM_CLASSES, 1], F32, tag="ciota", name="ciota")
            cg = cp.tile([128, NI4 * WT], BF16, tag="cg", name="cg")
            clab = cp.tile([1, BC], BF16, tag="clab", name="clab")
            ones10 = cp.tile([1, NUM_CLASSES], BF16, tag="ones10", name="ones10")
            h1all = cp.tile([128, 4 * BC], F8, tag="h1all", name="h1all")
            h2all = cp.tile([128, 2 * BC], F8, tag="h2all", name="h2all")
            # fe: plane-paired L4 rhs: cols 0:512 = feat, 512:1024 = embt (pad)
            fe = cp.tile([128, 2 * BC], F8, tag="fe", name="fe")
            h4all = cp.tile([128, 2 * BC], F8, tag="h4all", name="h4all")
            h5all = cp.tile([128, 4 * BC], F8, tag="h5all", name="h5all")
            onehot = cp.tile([NUM_CLASSES, BC], BF16, tag="onehot", name="onehot")

            # ---- const loads ----
            # encoder-critical first on sync (W6 streams queue behind them);
            # everything else on the scalar HWDGE queue so descriptor-gen and
            # wire time don't delay the encoder start.
            nc.sync.dma_start(cwenc[:], wenc_d[:])
            nc.sync.dma_start(cxre8[:], xre8_d[:])
            nc.sync.dma_start(cbias[:], bias_d[:])
            nc.scalar.dma_start(clab[:], lab_d[:])
            nc.scalar.dma_start(ciota[:], iota_d[:])
            nc.scalar.dma_start(cemb[:], emb_d[:])
            nc.scalar.dma_start(xre_b[:], xre_d[:])
            nc.scalar.dma_start(cg[:], g_d[:])
            nc.gpsimd.memset(ones10[:], 1.0)
            nc.gpsimd.memset(fe[:, BC:2 * BC], 0.0)

            def bslice(off):
                return cbias[:, off:off + 1]

            DR = mybir.MatmulPerfMode.DoubleRow

            def dr(out, lhs_base, lhs_off, lhs_stride, rhs_base, rhs_off,
                   rhs_stride, n, start, stop, m=128):
                """DoubleRow matmul from 2 stacked k-planes."""
                lhsT = bass_rust.AP(
                    tensor=lhs_base.tensor, offset=lhs_base.offset + lhs_off,
                    ap=[[lhs_base.ap[0][0], 128], [lhs_stride, 2], [1, m]],
                )
                rhs = bass_rust.AP(
                    tensor=rhs_base.tensor, offset=rhs_base.offset + rhs_off,
                    ap=[[rhs_base.ap[0][0], 128], [rhs_stride, 2], [1, n]],
                )
                nc.tensor.matmul(out, lhsT, rhs, start=start, stop=stop,
                                 perf_mode=DR)

            # ---- label one-hot + embedding (feature-major [16, BC]) ----
            psl = pm.tile([128, 512], F32, tag="ps", name="ps")
            nc.tensor.matmul(psl[0:NUM_CLASSES, 0:BC], ones10[:], clab[:],
                             start=True, stop=True)
            nc.vector.tensor_scalar(
                onehot[:], psl[0:NUM_CLASSES, 0:BC], ciota[:], None,
                mybir.AluOpType.is_equal,
            )
            pse = pm.tile([128, 512], F32, tag="ps", name="ps")
            nc.tensor.matmul(pse[0:LBL_DIM, 0:BC], cemb[:], onehot[:],
                             start=True, stop=True)
            nc.vector.tensor_copy(fe[0:LBL_DIM, BC:2 * BC], pse[0:LBL_DIM, 0:BC])

            # ---- encoder / decoder MLP (feature-major fp8, N = BC) ----
            wap_e = cwenc[:]
            x8ap = cxre8[:]
            h1ap = h1all[:]
            h2ap = h2all[:]
            feap = fe[:]
            h4ap = h4all[:]
            # L1: [600->512]: 3 DoubleRow passes (i4 pairs) + 1 plain (i4=6)
            for m in range(4):
                ps = pm.tile([128, 512], F32, tag="ps", name="ps")
                for j in range(3):
                    dr(ps[:, 0:BC], wap_e, OFF_W1 + 2 * j * 512 + m * 128, 512,
                       x8ap, 2 * j * 512, 512, BC, start=(j == 0), stop=False)
                nc.tensor.matmul(
                    ps[:, 0:BC],
                    cwenc[:, OFF_W1 + 6 * 512 + m * 128:OFF_W1 + 6 * 512 + (m + 1) * 128],
                    cxre8[:, 6 * 512:7 * 512],
                    start=False, stop=True,
                )
                h1m = h1all[:, m * BC:(m + 1) * BC]
                if m % 2 == 0:
                    nc.scalar.activation(h1m, ps[:, 0:BC], RELU, bias=bslice(OFF_B1 + m))
                else:
                    nc.vector.tensor_scalar(h1m, ps[:, 0:BC], bslice(OFF_B1 + m), 0.0, mybir.AluOpType.add, mybir.AluOpType.max)
            # L2: [512->256]: 2 DoubleRow passes
            for m in range(2):
                ps = pm.tile([128, 512], F32, tag="ps", name="ps")
                for j in range(2):
                    dr(ps[:, 0:BC], wap_e, OFF_W2 + 2 * j * 256 + m * 128, 256,
                       h1ap, 2 * j * BC, BC, BC, start=(j == 0), stop=(j == 1))
                h2m = h2all[:, m * BC:(m + 1) * BC]
                if m % 2 == 0:
                    nc.scalar.activation(h2m, ps[:, 0:BC], RELU, bias=bslice(OFF_B2 + m))
                else:
                    nc.vector.tensor_scalar(h2m, ps[:, 0:BC], bslice(OFF_B2 + m), 0.0, mybir.AluOpType.add, mybir.AluOpType.max)
            # L3: [256->128], no relu: 1 DoubleRow
            ps = pm.tile([128, 512], F32, tag="ps", name="ps")
            dr(ps[:, 0:BC], wap_e, OFF_W3, 128, h2ap, 0, BC, BC,
               start=True, stop=True)
            nc.vector.tensor_scalar(fe[:, 0:BC], ps[:, 0:BC], bslice(OFF_B3), None, mybir.AluOpType.add)
            # L4: [144->256]: 1 DoubleRow (planes: W4A/feat, W4B/embt)
            for m in range(2):
                ps = pm.tile([128, 512], F32, tag="ps", name="ps")
                dr(ps[:, 0:BC], wap_e, OFF_W4A + m * 128, OFF_W4B - OFF_W4A,
                   feap, 0, BC, BC, start=True, stop=True)
                h4m = h4all[:, m * BC:(m + 1) * BC]
                if m % 2 == 0:
                    nc.scalar.activation(h4m, ps[:, 0:BC], RELU, bias=bslice(OFF_B4 + m))
                else:
                    nc.vector.tensor_scalar(h4m, ps[:, 0:BC], bslice(OFF_B4 + m), 0.0, mybir.AluOpType.add, mybir.AluOpType.max)
            # L5: [256->512]: 1 DoubleRow per m
            for m in range(4):
                ps = pm.tile([128, 512], F32, tag="ps", name="ps")
                dr(ps[:, 0:BC], wap_e, OFF_W5 + m * 128, 512,
                   h4ap, 0, BC, BC, start=True, stop=True)
                h5m = h5all[:, m * BC:(m + 1) * BC]
                if m % 2 == 0:
                    nc.scalar.activation(h5m, ps[:, 0:BC], RELU, bias=bslice(OFF_B5 + m))
                else:
                    nc.vector.tensor_scalar(h5m, ps[:, 0:BC], bslice(OFF_B5 + m), 0.0, mybir.AluOpType.add, mybir.AluOpType.max)

            # ---- final layer + fused constraint epilogue ----
            # W6 streams in per-i4-block DMAs of [128, nwin*4*480] bf16;
            # outputs leave in per-(i4,bt) DMAs of [128, nwin*480] f32.
            def w6_block(i4):
                nwin = 4 if i4 < 6 else 1
                cols = nwin * 4 * WT
                t = wp.tile([128, 4 * 4 * WT], F8, tag="w6blk", name="w6blk", bufs=7)
                nc.sync.dma_start(t[:, 0:cols], w6_d[:, i4 * 4 * 4 * WT:i4 * 4 * 4 * WT + cols])
                return t

            hap = h5all[:]
            wblks = [w6_block(i4) for i4 in range(NI4)]
            for i4 in range(NI4):
                nwin = 4 if i4 < 6 else 1
                wblk = wblks[i4]
                ob = op.tile([128, NBT * 4 * WT], BF16, tag="ob", name="ob", bufs=3)
                obap = ob[:]
                for bt in range(NBT):
                    wap = wblk[:]
                    pss = []
                    for w in range(nwin):
                        # G first: it has no h5/W6 dependency, so the PE can
                        # run it while the decoder inputs are still in flight
                        ps = pm.tile([128, 512], F32, tag="ps", name="ps")[:, 0:WT]
                        pss.append(ps)
                        p0 = 32 * w
                        nc.tensor.matmul(
                            ps[:],
                            xre_b[p0:p0 + 32, i4 * 512 + bt * 128:i4 * 512 + (bt + 1) * 128],
                            cg[p0:p0 + 32, i4 * WT:(i4 + 1) * WT],
                            start=True, stop=False, tile_position=(p0, 0),
                        )
                    for w in range(nwin):
                        for kk in range(2):
                            # DoubleRow fp8: two k-tiles per pass
                            lhsT = bass_rust.AP(
                                tensor=hap.tensor,
                                offset=hap.offset + (2 * kk) * BC + bt * 128,
                                ap=[[hap.ap[0][0], 128], [BC, 2], [1, 128]],
                            )
                            rhs = bass_rust.AP(
                                tensor=wap.tensor,
                                offset=wap.offset + (w * 4 + 2 * kk) * WT,
                                ap=[[wap.ap[0][0], 128], [WT, 2], [1, WT]],
                            )
                            nc.tensor.matmul(
                                pss[w][:], lhsT, rhs,
                                start=False, stop=(kk == 1),
                                perf_mode=mybir.MatmulPerfMode.DoubleRow,
                            )
                    for w in range(nwin):
                        if (w + bt) % 2 == 0:
                            nc.vector.tensor_copy(ob[:, bt * 4 * WT + w * WT:bt * 4 * WT + (w + 1) * WT], pss[w][:])
                        else:
                            nc.scalar.copy(ob[:, bt * 4 * WT + w * WT:bt * 4 * WT + (w + 1) * WT], pss[w][:])
                # one DMA per i4 block covering all 4 batch tiles
                src = bass_rust.AP(
                    tensor=obap.tensor, offset=obap.offset,
                    ap=[[obap.ap[0][0], 128], [4 * WT, NBT], [1, nwin * WT]],
                )
                yap = y_d[:]
                dst = bass_rust.AP(
                    tensor=yap.tensor, offset=i4 * 4 * WT,
                    ap=[[D_OUT, 128], [128 * D_OUT, NBT], [1, nwin * WT]],
                )
                nc.sync.dma_start(dst, src)

    nc.compile()
    return nc


def _host_prep(inputs):
    """Build per-core in_maps from the full inputs."""
    x_full = np.asarray(inputs["low_res_data"], np.float32).reshape(B, D_IN)
    labels = np.asarray(inputs["labels"]).astype(np.float32)
    W1 = np.asarray(inputs["W1"], np.float32)
    W6 = np.asarray(inputs["W6"], np.float32)
    b6 = np.asarray(inputs["b6"], np.float32)

    # per-timestep blend coefficients (match the reference formulas)
    t = np.arange(HIGH_T)
    seg = np.clip(t // UP, 0, LOW_T - 2)
    alpha = ((t - seg * UP) / UP).astype(np.float64)
    is_anchor = (t % UP) == 0
    interior = t < (LOW_T - 1) * UP
    blendf = np.where(is_anchor, 1.0, np.where(interior, 0.8, 0.0))
    c_d = np.where(is_anchor, 0.0, np.where(interior, 0.2, 1.0))
    c_start = blendf * (1.0 - alpha)
    c_end = blendf * alpha

    # G matrix, window-blocked: [128, NI4*480]; window i lives at partition
    # offset 32*(i%4), col block i//4.  Rows r=0..29 <-> x col 24*i + r,
    # row 30 = bias row (paired with the constant-1.0 row of xre).
    gmat = np.zeros((128, NI4 * WT), np.float64)
    for tt in range(HIGH_T):
        i, dt = divmod(tt, 80)
        i4, wpos = divmod(i, 4)
        p0 = 32 * wpos
        sl = seg[tt] - 4 * i
        for f in range(FEAT):
            col = i4 * WT + FEAT * dt + f
            gmat[p0 + FEAT * sl + f, col] += c_start[tt]
            gmat[p0 + FEAT * (sl + 1) + f, col] += c_end[tt]
            gmat[p0 + 30, col] = c_d[tt] * np.float64(b6[FEAT * tt + f])
    gmat = gmat.astype(np.float32).astype(BF16_NP)

    c_d_full = np.repeat(c_d, FEAT).astype(np.float32)
    w6p = (W6 * c_d_full[None, :]).astype(F8_NP)     # [512, 12000]
    # repack: w6r[p, ((i*4 + k)*480 + c)] = w6p[k*128 + p, i*480 + c]
    w6r = np.ascontiguousarray(
        w6p.reshape(4, 128, NW, WT).transpose(1, 2, 0, 3).reshape(128, NW * 4 * WT)
    )

    # W1 rearranged to the window-blocked xre layout (rows 30/31 zero)
    w1re = np.zeros((128, NI4 * 512), np.float32)
    for c in range(D_IN):
        i, r = divmod(c, 24)
        i4, wpos = divmod(i, 4)
        w1re[32 * wpos + r, i4 * 512:(i4 + 1) * 512] = W1[c, :]
    # wenc pack [128, 6144] bf16
    wenc = np.zeros((128, WENC_COLS), np.float32)
    wenc[:, OFF_W1:OFF_W1 + NI4 * 512] = w1re
    W2 = np.asarray(inputs["W2"], np.float32)
    for k in range(4):
        wenc[:, OFF_W2 + k * 256:OFF_W2 + (k + 1) * 256] = W2[k * 128:(k + 1) * 128, :]
    W3 = np.asarray(inputs["W3"], np.float32)
    for k in range(2):
        wenc[:, OFF_W3 + k * 128:OFF_W3 + (k + 1) * 128] = W3[k * 128:(k + 1) * 128, :]
    W4 = np.asarray(inputs["W4"], np.float32)
    wenc[:, OFF_W4A:OFF_W4A + 256] = W4[:128]
    wenc[0:16, OFF_W4B:OFF_W4B + 256] = W4[128:144]
    W5 = np.asarray(inputs["W5"], np.float32)
    for k in range(2):
        wenc[:, OFF_W5 + k * 512:OFF_W5 + (k + 1) * 512] = W5[k * 128:(k + 1) * 128, :]
    wenc = wenc.astype(F8_NP)

    # bias pack [128, 13] f32
    bias = np.zeros((128, 13), np.float32)
    b = {k: np.asarray(inputs[k], np.float32) for k in ["b1", "b2", "b3", "b4", "b5"]}
    for m in range(4):
        bias[:, OFF_B1 + m] = b["b1"][m * 128:(m + 1) * 128]
        bias[:, OFF_B5 + m] = b["b5"][m * 128:(m + 1) * 128]
    for m in range(2):
        bias[:, OFF_B2 + m] = b["b2"][m * 128:(m + 1) * 128]
        bias[:, OFF_B4 + m] = b["b4"][m * 128:(m + 1) * 128]
    bias[:, OFF_B3] = b["b3"]

    const_map = {
        "wenc": wenc,
        "bias": bias,
        "w6r": w6r,
        "embT": np.asarray(inputs["emb"], np.float32).astype(BF16_NP),
        "iota10": np.arange(NUM_CLASSES, dtype=np.float32).reshape(NUM_CLASSES, 1),
        "gmat": gmat,
    }

    in_maps = []
    for c in range(NCORES):
        sl = slice(c * BC, (c + 1) * BC)
        xc = x_full[sl]                                   # [512, 600]
        # xre window-blocked transpose [128, NI4*512] bf16:
        # xre[32*w + r, i4*512 + bt*128 + j] = x[bt*128+j, 96*i4 + 24*w + r]
        # rows 30 = 1.0 (G bias row), 31 = 0.0; block 6 only has window 0.
        xre = np.zeros((128, NI4 * 512), np.float32)
        xb = xc.reshape(NBT, 128, D_IN)                   # [bt, j, c]
        for i4 in range(NI4):
            nwin = 4 if i4 < 6 else 1
            for w in range(nwin):
                c0 = 96 * i4 + 24 * w
                ncols = min(24 + 6, D_IN - c0) if i4 == 6 else 30
                # window rows r=0..29 come from x cols c0..c0+30 (next window
                # overlap); last window: cols 576..599 -> 24 rows, rest 0
                blk = xb[:, :, c0:c0 + ncols]             # [bt, j, r]
                xre[32 * w:32 * w + blk.shape[2], i4 * 512:(i4 + 1) * 512] = (
                    blk.transpose(2, 0, 1).reshape(blk.shape[2], BC)
                )
            xre[32 * np.arange(nwin) + 30, i4 * 512:(i4 + 1) * 512] = 1.0
        m = dict(const_map)
        m["xre"] = xre.astype(BF16_NP)
        m["xre8"] = xre.astype(F8_NP)
        m["labf"] = labels[sl].reshape(1, BC).astype(BF16_NP)
        in_maps.append(m)
    return in_maps


_NC_CACHE = None


def kernel(**inputs) -> np.ndarray:
    global _NC_CACHE
    if _NC_CACHE is None:
        _NC_CACHE = _build_nc()
    nc = _NC_CACHE
    in_maps = _host_prep(inputs)
    res = bass_utils.run_bass_kernel_spmd(nc, in_maps, core_ids=list(range(NCORES)))
    out = np.concatenate([res.results[c]["y"] for c in range(NCORES)], axis=0)
    return out.astype(np.float32).reshape(B, HIGH_T, FEAT)


# revision 20
# speedup vs baseline: 1.0319x; 1.0319x over previous
"""Trainium2 Bass kernel for nn_ConstrainedEnhancementModel.

Contract: kernel(**inputs) takes the FULL unsharded inputs (as produced by
reference.setup_inputs()) and returns the FULL [4096, 2000, 6] float32 output.

Strategy (pure data parallel over 8 NeuronCores, 512 batch rows each):
  - Feature-major MLP chain: every hidden activation is stored [feat, batch]
    so torch-layout weights [fan_in, fan_out] are directly the matmul lhsT.
  - The final layer flips to batch-major: lhsT = h5 (feature-major) slices,
    rhs = W6 tiles, so output DMA writes are contiguous.
  - The constraint/interpolation epilogue is folded into the final matmul:
        out = h5 @ (W6 * c_dec) + x @ G + ones * (b6 * c_dec)
    where G is a sparse constant [600, 12000] matrix holding the linear
    interpolation + anchor/blend coefficients.  G contributions are exact
    f32 (anchor timesteps reproduce the input bit-exactly); the decoded
    path is bf16 (it only ever enters scaled by 0.2 or in the tail).
  - DMA instruction count is minimized (each DMA_DIRECT2D costs ~600ns of
    serialized descriptor-gen on its issuing engine queue): weights/biases
    are host-packed into a handful of wide tensors, W6 streams in 7 block
    DMAs, outputs leave in 28 block DMAs of [128, 1920] f32.
  - x is host-transposed into the window-blocked xre layout (pure layout
    transform), so no on-device transposes are needed.
"""

import numpy as np
import ml_dtypes

import bass_rust
import concourse.bass as bass
import concourse.bacc as bacc
import concourse.mybir as mybir
import concourse.tile as tile
from concourse import bass_utils

F32 = mybir.dt.float32
BF16 = mybir.dt.bfloat16
BF16_NP = ml_dtypes.bfloat16
F8 = mybir.dt.float8e4
F8_NP = ml_dtypes.float8_e4m3

# Problem config (hardcoded; must match the reference)
LOW_T = 100
HIGH_T = 2000
FEAT = 6
HID = 256
NUM_CLASSES = 10
LBL_DIM = 16
UP = 20
B = 4096
NCORES = 8
BC = B // NCORES          # 512 batch rows per core
NBT = BC // 128           # 4 batch tiles per core
D_IN = LOW_T * FEAT       # 600
D_OUT = HIGH_T * FEAT     # 12000
NW = 25                   # output windows (80 timesteps * 6 feats = 480 cols)
WT = 480
NI4 = 7                   # ceil(25/4) groups of 4 windows

# packed encoder-weight column offsets in wenc [128, 6400] fp8
OFF_W1 = 0          # 7 blocks of 512
OFF_W2 = 3584       # 4 k-tiles of 256
OFF_W3 = 4608       # 2 k-tiles of 128
OFF_W4A = 4864      # 256
OFF_W5 = 5120       # 2 k-tiles of 512
OFF_W4B = 6144      # 256 (rows 0-15 real, rest zero; plane-paired with W4A)
WENC_COLS = 6400
# packed bias column offsets in bias [128, 13] f32
OFF_B1, OFF_B2, OFF_B3, OFF_B4, OFF_B5 = 0, 4, 6, 7, 9


def _build_nc():
    """Build the single-core Bass program (SPMD: same program on all 8)."""
    nc = bacc.Bacc("TRN2", target_bir_lowering=False, debug=False)

    xre_d = nc.dram_tensor("xre", [128, NI4 * 512], BF16, kind="ExternalInput")
    xre8_d = nc.dram_tensor("xre8", [128, NI4 * 512], F8, kind="ExternalInput")
    lab_d = nc.dram_tensor("labf", [1, BC], BF16, kind="ExternalInput")
    wenc_d = nc.dram_tensor("wenc", [128, WENC_COLS], F8, kind="ExternalInput")
    bias_d = nc.dram_tensor("bias", [128, 13], F32, kind="ExternalInput")
    emb_d = nc.dram_tensor("embT", [NUM_CLASSES, LBL_DIM], BF16, kind="ExternalInput")
    iota_d = nc.dram_tensor("iota10", [NUM_CLASSES, 1], F32, kind="ExternalInput")
    g_d = nc.dram_tensor("gmat", [128, NI4 * WT], BF16, kind="ExternalInput")
    w6_d = nc.dram_tensor("w6r", [128, NW * 4 * WT], F8, kind="ExternalInput")
    y_d = nc.dram_tensor("y", [BC, D_OUT], BF16, kind="ExternalOutput")

    RELU = mybir.ActivationFunctionType.Relu

    with tile.TileContext(nc) as tc:
        with (
            tc.tile_pool(name="const", bufs=1) as cp,
            tc.tile_pool(name="w6pool", bufs=2) as wp,
            tc.tile_pool(name="outpool", bufs=4) as op,
            tc.tile_pool(name="ppool", bufs=8, space="PSUM") as pm,
        ):
            # ---- persistent SBUF tensors ----
            xre_b = cp.tile([128, NI4 * 512], BF16, tag="xre_b", name="xre_b")
            cxre8 = cp.tile([128, NI4 * 512], F8, tag="cxre8", name="cxre8")
            cwenc = cp.tile([128, WENC_COLS], F8, tag="cwenc", name="cwenc")
            cbias = cp.tile([128, 13], F32, tag="cbias", name="cbias")
            cemb = cp.tile([NUM_CLASSES, LBL_DIM], BF16, tag="cemb", name="cemb")
            ciota = cp.tile([NUM_CLASSES, 1], F32, tag="ciota", name="ciota")
            cg = cp.tile([128, NI4 * WT], BF16, tag="cg", name="cg")
            clab = cp.tile([1, BC], BF16, tag="clab", name="clab")
            ones10 = cp.tile([1, NUM_CLASSES], BF16, tag="ones10", name="ones10")
            h1all = cp.tile([128, 4 * BC], F8, tag="h1all", name="h1all")
            h2all = cp.tile([128, 2 * BC], F8, tag="h2all", name="h2all")
            # fe: plane-paired L4 rhs: cols 0:512 = feat, 512:1024 = embt (pad)
            fe = cp.tile([128, 2 * BC], F8, tag="fe", name="fe")
            h4all = cp.tile([128, 2 * BC], F8, tag="h4all", name="h4all")
            h5all = cp.tile([128, 4 * BC], F8, tag="h5all", name="h5all")
            onehot = cp.tile([NUM_CLASSES, BC], BF16, tag="onehot", name="onehot")

            # ---- const loads ----
            # encoder-critical first on sync (W6 streams queue behind them);
            # everything else on the scalar HWDGE queue so descriptor-gen and
            # wire time don't delay the encoder start.
            nc.sync.dma_start(cwenc[:], wenc_d[:])
            nc.sync.dma_start(cxre8[:], xre8_d[:])
            nc.sync.dma_start(cbias[:], bias_d[:])
            nc.scalar.dma_start(clab[:], lab_d[:])
            nc.scalar.dma_start(ciota[:], iota_d[:])
            nc.scalar.dma_start(cemb[:], emb_d[:])
            nc.scalar.dma_start(xre_b[:], xre_d[:])
            nc.scalar.dma_start(cg[:], g_d[:])
            nc.gpsimd.memset(ones10[:], 1.0)
            nc.gpsimd.memset(fe[:, BC:2 * BC], 0.0)

            def bslice(off):
                return cbias[:, off:off + 1]

            DR = mybir.MatmulPerfMode.DoubleRow

            def dr(out, lhs_base, lhs_off, lhs_stride, rhs_base, rhs_off,
                   rhs_stride, n, start, stop, m=128):
                """DoubleRow matmul from 2 stacked k-planes."""
                lhsT = bass_rust.AP(
                    tensor=lhs_base.tensor, offset=lhs_base.offset + lhs_off,
                    ap=[[lhs_base.ap[0][0], 128], [lhs_stride, 2], [1, m]],
                )
                rhs = bass_rust.AP(
                    tensor=rhs_base.tensor, offset=rhs_base.offset + rhs_off,
                    ap=[[rhs_base.ap[0][0], 128], [rhs_stride, 2], [1, n]],
                )
                nc.tensor.matmul(out, lhsT, rhs, start=start, stop=stop,
                                 perf_mode=DR)

            # ---- label one-hot + embedding (feature-major [16, BC]) ----
            psl = pm.tile([128, 512], F32, tag="ps", name="ps")
            nc.tensor.matmul(psl[0:NUM_CLASSES, 0:BC], ones10[:], clab[:],
                             start=True, stop=True)
            nc.vector.tensor_scalar(
                onehot[:], psl[0:NUM_CLASSES, 0:BC], ciota[:], None,
                mybir.AluOpType.is_equal,
            )
            pse = pm.tile([128, 512], F32, tag="ps", name="ps")
            nc.tensor.matmul(pse[0:LBL_DIM, 0:BC], cemb[:], onehot[:],
                             start=True, stop=True)
            nc.vector.tensor_copy(fe[0:LBL_DIM, BC:2 * BC], pse[0:LBL_DIM, 0:BC])

            # ---- encoder / decoder MLP (feature-major fp8, N = BC) ----
            wap_e = cwenc[:]
            x8ap = cxre8[:]
            h1ap = h1all[:]
            h2ap = h2all[:]
            feap = fe[:]
            h4ap = h4all[:]
            # L1: [600->512]: 3 DoubleRow passes (i4 pairs) + 1 plain (i4=6)
            for m in range(4):
                ps = pm.tile([128, 512], F32, tag="ps", name="ps")
                for j in range(3):
                    dr(ps[:, 0:BC], wap_e, OFF_W1 + 2 * j * 512 + m * 128, 512,
                       x8ap, 2 * j * 512, 512, BC, start=(j == 0), stop=False)
                nc.tensor.matmul(
                    ps[:, 0:BC],
                    cwenc[:, OFF_W1 + 6 * 512 + m * 128:OFF_W1 + 6 * 512 + (m + 1) * 128],
                    cxre8[:, 6 * 512:7 * 512],
                    start=False, stop=True,
                )
                h1m = h1all[:, m * BC:(m + 1) * BC]
                if m % 2 == 0:
                    nc.scalar.activation(h1m, ps[:, 0:BC], RELU, bias=bslice(OFF_B1 + m))
                else:
                    nc.vector.tensor_scalar(h1m, ps[:, 0:BC], bslice(OFF_B1 + m), 0.0, mybir.AluOpType.add, mybir.AluOpType.max)
            # L2: [512->256]: 2 DoubleRow passes
            for m in range(2):
                ps = pm.tile([128, 512], F32, tag="ps", name="ps")
                for j in range(2):
                    dr(ps[:, 0:BC], wap_e, OFF_W2 + 2 * j * 256 + m * 128, 256,
                       h1ap, 2 * j * BC, BC, BC, start=(j == 0), stop=(j == 1))
                h2m = h2all[:, m * BC:(m + 1) * BC]
                if m % 2 == 0:
                    nc.scalar.activation(h2m, ps[:, 0:BC], RELU, bias=bslice(OFF_B2 + m))
                else:
                    nc.vector.tensor_scalar(h2m, ps[:, 0:BC], bslice(OFF_B2 + m), 0.0, mybir.AluOpType.add, mybir.AluOpType.max)
            # L3: [256->128], no relu: 1 DoubleRow
            ps = pm.tile([128, 512], F32, tag="ps", name="ps")
            dr(ps[:, 0:BC], wap_e, OFF_W3, 128, h2ap, 0, BC, BC,
               start=True, stop=True)
            nc.vector.tensor_scalar(fe[:, 0:BC], ps[:, 0:BC], bslice(OFF_B3), None, mybir.AluOpType.add)
            # L4: [144->256]: 1 DoubleRow (planes: W4A/feat, W4B/embt)
            for m in range(2):
                ps = pm.tile([128, 512], F32, tag="ps", name="ps")
                dr(ps[:, 0:BC], wap_e, OFF_W4A + m * 128, OFF_W4B - OFF_W4A,
                   feap, 0, BC, BC, start=True, stop=True)
                h4m = h4all[:, m * BC:(m + 1) * BC]
                if m % 2 == 0:
                    nc.scalar.activation(h4m, ps[:, 0:BC], RELU, bias=bslice(OFF_B4 + m))
                else:
                    nc.vector.tensor_scalar(h4m, ps[:, 0:BC], bslice(OFF_B4 + m), 0.0, mybir.AluOpType.add, mybir.AluOpType.max)
            # L5: [256->512]: 1 DoubleRow per m
            for m in range(4):
                ps = pm.tile([128, 512], F32, tag="ps", name="ps")
                dr(ps[:, 0:BC], wap_e, OFF_W5 + m * 128, 512,
                   h4ap, 0, BC, BC, start=True, stop=True)
                h5m = h5all[:, m * BC:(m + 1) * BC]
                if m % 2 == 0:
                    nc.scalar.activation(h5m, ps[:, 0:BC], RELU, bias=bslice(OFF_B5 + m))
                else:
                    nc.vector.tensor_scalar(h5m, ps[:, 0:BC], bslice(OFF_B5 + m), 0.0, mybir.AluOpType.add, mybir.AluOpType.max)

            # ---- final layer + fused constraint epilogue ----
            # W6 streams in per-i4-block DMAs of [128, nwin*4*480] bf16;
            # outputs leave in per-(i4,bt) DMAs of [128, nwin*480] f32.
            def w6_block(i4):
                nwin = 4 if i4 < 6 else 1
                cols = nwin * 4 * WT
                t = wp.tile([128, 4 * 4 * WT], F8, tag="w6blk", name="w6blk", bufs=7)
                nc.sync.dma_start(t[:, 0:cols], w6_d[:, i4 * 4 * 4 * WT:i4 * 4 * 4 * WT + cols])
                return t

            hap = h5all[:]
            wblks = [w6_block(i4) for i4 in range(NI4)]
            for i4 in range(NI4):
                nwin = 4 if i4 < 6 else 1
                wblk = wblks[i4]
                for bt in range(NBT):
                    ob = op.tile([128, 4 * WT], BF16, tag="ob", name="ob", bufs=8)
                    wap = wblk[:]
                    pss = []
                    for w in range(nwin):
                        # G first: it has no h5/W6 dependency, so the PE can
                        # run it while the decoder inputs are still in flight
                        ps = pm.tile([128, 512], F32, tag="ps", name="ps")[:, 0:WT]
                        pss.append(ps)
                        p0 = 32 * w
                        nc.tensor.matmul(
                            ps[:],
                            xre_b[p0:p0 + 32, i4 * 512 + bt * 128:i4 * 512 + (bt + 1) * 128],
                            cg[p0:p0 + 32, i4 * WT:(i4 + 1) * WT],
                            start=True, stop=False, tile_position=(p0, 0),
                        )
                    for w in range(nwin):
                        for kk in range(2):
                            # DoubleRow fp8: two k-tiles per pass
                            lhsT = bass_rust.AP(
                                tensor=hap.tensor,
                                offset=hap.offset + (2 * kk) * BC + bt * 128,
                                ap=[[hap.ap[0][0], 128], [BC, 2], [1, 128]],
                            )
                            rhs = bass_rust.AP(
                                tensor=wap.tensor,
                                offset=wap.offset + (w * 4 + 2 * kk) * WT,
                                ap=[[wap.ap[0][0], 128], [WT, 2], [1, WT]],
                            )
                            nc.tensor.matmul(
                                pss[w][:], lhsT, rhs,
                                start=False, stop=(kk == 1),
                                perf_mode=mybir.MatmulPerfMode.DoubleRow,
                            )
                    for w in range(nwin):
                        if (w + bt) % 2 == 0:
                            nc.vector.tensor_copy(ob[:, w * WT:(w + 1) * WT], pss[w][:])
                        else:
                            nc.scalar.copy(ob[:, w * WT:(w + 1) * WT], pss[w][:])
                    nc.sync.dma_start(
                        y_d[bt * 128:(bt + 1) * 128, i4 * 4 * WT:i4 * 4 * WT + nwin * WT],
                        ob[:, 0:nwin * WT],
                    )


    nc.compile()
    return nc


def _host_prep(inputs):
    """Build per-core in_maps from the full inputs."""
    x_full = np.asarray(inputs["low_res_data"], np.float32).reshape(B, D_IN)
    labels = np.asarray(inputs["labels"]).astype(np.float32)
    W1 = np.asarray(inputs["W1"], np.float32)
    W6 = np.asarray(inputs["W6"], np.float32)
    b6 = np.asarray(inputs["b6"], np.float32)

    # per-timestep blend coefficients (match the reference formulas)
    t = np.arange(HIGH_T)
    seg = np.clip(t // UP, 0, LOW_T - 2)
    alpha = ((t - seg * UP) / UP).astype(np.float64)
    is_anchor = (t % UP) == 0
    interior = t < (LOW_T - 1) * UP
    blendf = np.where(is_anchor, 1.0, np.where(interior, 0.8, 0.0))
    c_d = np.where(is_anchor, 0.0, np.where(interior, 0.2, 1.0))
    c_start = blendf * (1.0 - alpha)
    c_end = blendf * alpha

    # G matrix, window-blocked: [128, NI4*480]; window i lives at partition
    # offset 32*(i%4), col block i//4.  Rows r=0..29 <-> x col 24*i + r,
    # row 30 = bias row (paired with the constant-1.0 row of xre).
    gmat = np.zeros((128, NI4 * WT), np.float64)
    for tt in range(HIGH_T):
        i, dt = divmod(tt, 80)
        i4, wpos = divmod(i, 4)
        p0 = 32 * wpos
        sl = seg[tt] - 4 * i
        for f in range(FEAT):
            col = i4 * WT + FEAT * dt + f
            gmat[p0 + FEAT * sl + f, col] += c_start[tt]
            gmat[p0 + FEAT * (sl + 1) + f, col] += c_end[tt]
            gmat[p0 + 30, col] = c_d[tt] * np.float64(b6[FEAT * tt + f])
    gmat = gmat.astype(np.float32).astype(BF16_NP)

    c_d_full = np.repeat(c_d, FEAT).astype(np.float32)
    w6p = (W6 * c_d_full[None, :]).astype(F8_NP)     # [512, 12000]
    # repack: w6r[p, ((i*4 + k)*480 + c)] = w6p[k*128 + p, i*480 + c]
    w6r = np.ascontiguousarray(
        w6p.reshape(4, 128, NW, WT).transpose(1, 2, 0, 3).reshape(128, NW * 4 * WT)
    )

    # W1 rearranged to the window-blocked xre layout (rows 30/31 zero)
    w1re = np.zeros((128, NI4 * 512), np.float32)
    for c in range(D_IN):
        i, r = divmod(c, 24)
        i4, wpos = divmod(i, 4)
        w1re[32 * wpos + r, i4 * 512:(i4 + 1) * 512] = W1[c, :]
    # wenc pack [128, 6144] bf16
    wenc = np.zeros((128, WENC_COLS), np.float32)
    wenc[:, OFF_W1:OFF_W1 + NI4 * 512] = w1re
    W2 = np.asarray(inputs["W2"], np.float32)
    for k in range(4):
        wenc[:, OFF_W2 + k * 256:OFF_W2 + (k + 1) * 256] = W2[k * 128:(k + 1) * 128, :]
    W3 = np.asarray(inputs["W3"], np.float32)
    for k in range(2):
        wenc[:, OFF_W3 + k * 128:OFF_W3 + (k + 1) * 128] = W3[k * 128:(k + 1) * 128, :]
    W4 = np.asarray(inputs["W4"], np.float32)
    wenc[:, OFF_W4A:OFF_W4A + 256] = W4[:128]
    wenc[0:16, OFF_W4B:OFF_W4B + 256] = W4[128:144]
    W5 = np.asarray(inputs["W5"], np.float32)
    for k in range(2):
        wenc[:, OFF_W5 + k * 512:OFF_W5 + (k + 1) * 512] = W5[k * 128:(k + 1) * 128, :]
    wenc = wenc.astype(F8_NP)

    # bias pack [128, 13] f32
    bias = np.zeros((128, 13), np.float32)
    b = {k: np.asarray(inputs[k], np.float32) for k in ["b1", "b2", "b3", "b4", "b5"]}
    for m in range(4):
        bias[:, OFF_B1 + m] = b["b1"][m * 128:(m + 1) * 128]
        bias[:, OFF_B5 + m] = b["b5"][m * 128:(m + 1) * 128]
    for m in range(2):
        bias[:, OFF_B2 + m] = b["b2"][m * 128:(m + 1) * 128]
        bias[:, OFF_B4 + m] = b["b4"][m * 128:(m + 1) * 128]
    bias[:, OFF_B3] = b["b3"]

    const_map = {
        "wenc": wenc,
        "bias": bias,
        "w6r": w6r,
        "embT": np.asarray(inputs["emb"], np.float32).astype(BF16_NP),
        "iota10": np.arange(NUM_CLASSES, dtype=np.float32).reshape(NUM_CLASSES, 1),
        "gmat": gmat,
    }

    in_maps = []
    for c in range(NCORES):
        sl = slice(c * BC, (c + 1) * BC)
        xc = x_full[sl]                                   # [512, 600]
        # xre window-blocked transpose [128, NI4*512] bf16:
        # xre[32*w + r, i4*512 + bt*128 + j] = x[bt*128+j, 96*i4 + 24*w + r]
        # rows 30 = 1.0 (G bias row), 31 = 0.0; block 6 only has window 0.
        xre = np.zeros((128, NI4 * 512), np.float32)
        xb = xc.reshape(NBT, 128, D_IN)                   # [bt, j, c]
        for i4 in range(NI4):
            nwin = 4 if i4 < 6 else 1
            for w in range(nwin):
                c0 = 96 * i4 + 24 * w
                ncols = min(24 + 6, D_IN - c0) if i4 == 6 else 30
                # window rows r=0..29 come from x cols c0..c0+30 (next window
                # overlap); last window: cols 576..599 -> 24 rows, rest 0
                blk = xb[:, :, c0:c0 + ncols]             # [bt, j, r]
                xre[32 * w:32 * w + blk.shape[2], i4 * 512:(i4 + 1) * 512] = (
                    blk.transpose(2, 0, 1).reshape(blk.shape[2], BC)
                )
            xre[32 * np.arange(nwin) + 30, i4 * 512:(i4 + 1) * 512] = 1.0
        m = dict(const_map)
        m["xre"] = xre.astype(BF16_NP)
        m["xre8"] = xre.astype(F8_NP)
        m["labf"] = labels[sl].reshape(1, BC).astype(BF16_NP)
        in_maps.append(m)
    return in_maps


_NC_CACHE = None


def kernel(**inputs) -> np.ndarray:
    global _NC_CACHE
    if _NC_CACHE is None:
        _NC_CACHE = _build_nc()
    nc = _NC_CACHE
    in_maps = _host_prep(inputs)
    res = bass_utils.run_bass_kernel_spmd(nc, in_maps, core_ids=list(range(NCORES)))
    out = np.concatenate([res.results[c]["y"] for c in range(NCORES)], axis=0)
    return out.astype(np.float32).reshape(B, HIGH_T, FEAT)


# revision 21
# speedup vs baseline: 1.0432x; 1.0109x over previous
"""Trainium2 Bass kernel for nn_ConstrainedEnhancementModel.

Contract: kernel(**inputs) takes the FULL unsharded inputs (as produced by
reference.setup_inputs()) and returns the FULL [4096, 2000, 6] float32 output.

Strategy (pure data parallel over 8 NeuronCores, 512 batch rows each):
  - Feature-major MLP chain: every hidden activation is stored [feat, batch]
    so torch-layout weights [fan_in, fan_out] are directly the matmul lhsT.
  - The final layer flips to batch-major: lhsT = h5 (feature-major) slices,
    rhs = W6 tiles, so output DMA writes are contiguous.
  - The constraint/interpolation epilogue is folded into the final matmul:
        out = h5 @ (W6 * c_dec) + x @ G + ones * (b6 * c_dec)
    where G is a sparse constant [600, 12000] matrix holding the linear
    interpolation + anchor/blend coefficients.  G contributions are exact
    f32 (anchor timesteps reproduce the input bit-exactly); the decoded
    path is bf16 (it only ever enters scaled by 0.2 or in the tail).
  - DMA instruction count is minimized (each DMA_DIRECT2D costs ~600ns of
    serialized descriptor-gen on its issuing engine queue): weights/biases
    are host-packed into a handful of wide tensors, W6 streams in 7 block
    DMAs, outputs leave in 28 block DMAs of [128, 1920] f32.
  - x is host-transposed into the window-blocked xre layout (pure layout
    transform), so no on-device transposes are needed.
"""

import numpy as np
import ml_dtypes

import bass_rust
import concourse.bass as bass
import concourse.bacc as bacc
import concourse.mybir as mybir
import concourse.tile as tile
from concourse import bass_utils

F32 = mybir.dt.float32
BF16 = mybir.dt.bfloat16
BF16_NP = ml_dtypes.bfloat16
F8 = mybir.dt.float8e4
F8_NP = ml_dtypes.float8_e4m3

# Problem config (hardcoded; must match the reference)
LOW_T = 100
HIGH_T = 2000
FEAT = 6
HID = 256
NUM_CLASSES = 10
LBL_DIM = 16
UP = 20
B = 4096
NCORES = 8
BC = B // NCORES          # 512 batch rows per core
NBT = BC // 128           # 4 batch tiles per core
D_IN = LOW_T * FEAT       # 600
D_OUT = HIGH_T * FEAT     # 12000
NW = 25                   # output windows (80 timesteps * 6 feats = 480 cols)
WT = 480
NI4 = 7                   # ceil(25/4) groups of 4 windows

# packed encoder-weight column offsets in wenc [128, 6400] fp8
OFF_W1 = 0          # 7 blocks of 512
OFF_W2 = 3584       # 4 k-tiles of 256
OFF_W3 = 4608       # 2 k-tiles of 128
OFF_W4A = 4864      # 256
OFF_W5 = 5120       # 2 k-tiles of 512
OFF_W4B = 6144      # 256 (rows 0-15 real, rest zero; plane-paired with W4A)
WENC_COLS = 6400
# packed bias column offsets in bias [128, 13] f32
OFF_B1, OFF_B2, OFF_B3, OFF_B4, OFF_B5 = 0, 4, 6, 7, 9


def _build_nc():
    """Build the single-core Bass program (SPMD: same program on all 8)."""
    nc = bacc.Bacc("TRN2", target_bir_lowering=False, debug=False)

    xre_d = nc.dram_tensor("xre", [128, NI4 * 512], BF16, kind="ExternalInput")
    xre8_d = nc.dram_tensor("xre8", [128, NI4 * 512], F8, kind="ExternalInput")
    lab_d = nc.dram_tensor("labf", [1, BC], BF16, kind="ExternalInput")
    wenc_d = nc.dram_tensor("wenc", [128, WENC_COLS], F8, kind="ExternalInput")
    bias_d = nc.dram_tensor("bias", [128, 13], F32, kind="ExternalInput")
    emb_d = nc.dram_tensor("embT", [NUM_CLASSES, LBL_DIM], BF16, kind="ExternalInput")
    iota_d = nc.dram_tensor("iota10", [NUM_CLASSES, 1], F32, kind="ExternalInput")
    g_d = nc.dram_tensor("gmat", [128, NI4 * WT], BF16, kind="ExternalInput")
    w6_d = nc.dram_tensor("w6r", [128, NW * 4 * WT], F8, kind="ExternalInput")
    y_d = nc.dram_tensor("y", [BC, D_OUT], BF16, kind="ExternalOutput")

    RELU = mybir.ActivationFunctionType.Relu

    with tile.TileContext(nc) as tc:
        with (
            tc.tile_pool(name="const", bufs=1) as cp,
            tc.tile_pool(name="w6pool", bufs=2) as wp,
            tc.tile_pool(name="outpool", bufs=4) as op,
            tc.tile_pool(name="ppool", bufs=8, space="PSUM") as pm,
        ):
            # ---- persistent SBUF tensors ----
            xre_b = cp.tile([128, NI4 * 512], BF16, tag="xre_b", name="xre_b")
            cxre8 = cp.tile([128, NI4 * 512], F8, tag="cxre8", name="cxre8")
            cwenc = cp.tile([128, WENC_COLS], F8, tag="cwenc", name="cwenc")
            cbias = cp.tile([128, 13], F32, tag="cbias", name="cbias")
            cemb = cp.tile([NUM_CLASSES, LBL_DIM], BF16, tag="cemb", name="cemb")
            ciota = cp.tile([NUM_CLASSES, 1], F32, tag="ciota", name="ciota")
            cg = cp.tile([128, NI4 * WT], BF16, tag="cg", name="cg")
            clab = cp.tile([1, BC], BF16, tag="clab", name="clab")
            ones10 = cp.tile([1, NUM_CLASSES], BF16, tag="ones10", name="ones10")
            h1all = cp.tile([128, 4 * BC], F8, tag="h1all", name="h1all")
            h2all = cp.tile([128, 2 * BC], F8, tag="h2all", name="h2all")
            # fe: plane-paired L4 rhs: cols 0:512 = feat, 512:1024 = embt (pad)
            fe = cp.tile([128, 2 * BC], F8, tag="fe", name="fe")
            h4all = cp.tile([128, 2 * BC], F8, tag="h4all", name="h4all")
            h5all = cp.tile([128, 4 * BC], F8, tag="h5all", name="h5all")
            onehot = cp.tile([NUM_CLASSES, BC], BF16, tag="onehot", name="onehot")

            # ---- const loads ----
            # encoder-critical first on sync (W6 streams queue behind them);
            # everything else on the scalar HWDGE queue so descriptor-gen and
            # wire time don't delay the encoder start.
            nc.sync.dma_start(cwenc[:], wenc_d[:])
            nc.sync.dma_start(cxre8[:], xre8_d[:])
            nc.sync.dma_start(cbias[:], bias_d[:])
            nc.scalar.dma_start(clab[:], lab_d[:])
            nc.scalar.dma_start(ciota[:], iota_d[:])
            nc.scalar.dma_start(cemb[:], emb_d[:])
            nc.scalar.dma_start(xre_b[:], xre_d[:])
            nc.scalar.dma_start(cg[:], g_d[:])
            nc.gpsimd.memset(ones10[:], 1.0)
            nc.gpsimd.memset(fe[:, BC:2 * BC], 0.0)

            def bslice(off):
                return cbias[:, off:off + 1]

            DR = mybir.MatmulPerfMode.DoubleRow

            def dr(out, lhs_base, lhs_off, lhs_stride, rhs_base, rhs_off,
                   rhs_stride, n, start, stop, m=128):
                """DoubleRow matmul from 2 stacked k-planes."""
                lhsT = bass_rust.AP(
                    tensor=lhs_base.tensor, offset=lhs_base.offset + lhs_off,
                    ap=[[lhs_base.ap[0][0], 128], [lhs_stride, 2], [1, m]],
                )
                rhs = bass_rust.AP(
                    tensor=rhs_base.tensor, offset=rhs_base.offset + rhs_off,
                    ap=[[rhs_base.ap[0][0], 128], [rhs_stride, 2], [1, n]],
                )
                nc.tensor.matmul(out, lhsT, rhs, start=start, stop=stop,
                                 perf_mode=DR)

            # ---- label one-hot + embedding (feature-major [16, BC]) ----
            psl = pm.tile([128, 512], F32, tag="ps", name="ps")
            nc.tensor.matmul(psl[0:NUM_CLASSES, 0:BC], ones10[:], clab[:],
                             start=True, stop=True)
            nc.vector.tensor_scalar(
                onehot[:], psl[0:NUM_CLASSES, 0:BC], ciota[:], None,
                mybir.AluOpType.is_equal,
            )
            pse = pm.tile([128, 512], F32, tag="ps", name="ps")
            nc.tensor.matmul(pse[0:LBL_DIM, 0:BC], cemb[:], onehot[:],
                             start=True, stop=True)
            nc.vector.tensor_copy(fe[0:LBL_DIM, BC:2 * BC], pse[0:LBL_DIM, 0:BC])

            # ---- encoder / decoder MLP (feature-major fp8, N = BC) ----
            wap_e = cwenc[:]
            x8ap = cxre8[:]
            h1ap = h1all[:]
            h2ap = h2all[:]
            feap = fe[:]
            h4ap = h4all[:]
            # L1: [600->512]: 3 DoubleRow passes (i4 pairs) + 1 plain (i4=6)
            for m in range(4):
                ps = pm.tile([128, 512], F32, tag="ps", name="ps")
                for j in range(3):
                    dr(ps[:, 0:BC], wap_e, OFF_W1 + 2 * j * 512 + m * 128, 512,
                       x8ap, 2 * j * 512, 512, BC, start=(j == 0), stop=False)
                nc.tensor.matmul(
                    ps[:, 0:BC],
                    cwenc[:, OFF_W1 + 6 * 512 + m * 128:OFF_W1 + 6 * 512 + (m + 1) * 128],
                    cxre8[:, 6 * 512:7 * 512],
                    start=False, stop=True,
                )
                h1m = h1all[:, m * BC:(m + 1) * BC]
                if m % 2 == 0:
                    nc.scalar.activation(h1m, ps[:, 0:BC], RELU, bias=bslice(OFF_B1 + m))
                else:
                    nc.vector.tensor_scalar(h1m, ps[:, 0:BC], bslice(OFF_B1 + m), 0.0, mybir.AluOpType.add, mybir.AluOpType.max)
            # L2: [512->256]: 2 DoubleRow passes
            for m in range(2):
                ps = pm.tile([128, 512], F32, tag="ps", name="ps")
                for j in range(2):
                    dr(ps[:, 0:BC], wap_e, OFF_W2 + 2 * j * 256 + m * 128, 256,
                       h1ap, 2 * j * BC, BC, BC, start=(j == 0), stop=(j == 1))
                h2m = h2all[:, m * BC:(m + 1) * BC]
                if m % 2 == 0:
                    nc.scalar.activation(h2m, ps[:, 0:BC], RELU, bias=bslice(OFF_B2 + m))
                else:
                    nc.vector.tensor_scalar(h2m, ps[:, 0:BC], bslice(OFF_B2 + m), 0.0, mybir.AluOpType.add, mybir.AluOpType.max)
            # L3: [256->128], no relu: 1 DoubleRow
            ps = pm.tile([128, 512], F32, tag="ps", name="ps")
            dr(ps[:, 0:BC], wap_e, OFF_W3, 128, h2ap, 0, BC, BC,
               start=True, stop=True)
            nc.vector.tensor_scalar(fe[:, 0:BC], ps[:, 0:BC], bslice(OFF_B3), None, mybir.AluOpType.add)
            # L4: [144->256]: 1 DoubleRow (planes: W4A/feat, W4B/embt)
            for m in range(2):
                ps = pm.tile([128, 512], F32, tag="ps", name="ps")
                dr(ps[:, 0:BC], wap_e, OFF_W4A + m * 128, OFF_W4B - OFF_W4A,
                   feap, 0, BC, BC, start=True, stop=True)
                h4m = h4all[:, m * BC:(m + 1) * BC]
                if m % 2 == 0:
                    nc.scalar.activation(h4m, ps[:, 0:BC], RELU, bias=bslice(OFF_B4 + m))
                else:
                    nc.vector.tensor_scalar(h4m, ps[:, 0:BC], bslice(OFF_B4 + m), 0.0, mybir.AluOpType.add, mybir.AluOpType.max)
            # L5: [256->512]: 1 DoubleRow per m
            for m in range(4):
                ps = pm.tile([128, 512], F32, tag="ps", name="ps")
                dr(ps[:, 0:BC], wap_e, OFF_W5 + m * 128, 512,
                   h4ap, 0, BC, BC, start=True, stop=True)
                h5m = h5all[:, m * BC:(m + 1) * BC]
                if m % 2 == 0:
                    nc.scalar.activation(h5m, ps[:, 0:BC], RELU, bias=bslice(OFF_B5 + m))
                else:
                    nc.vector.tensor_scalar(h5m, ps[:, 0:BC], bslice(OFF_B5 + m), 0.0, mybir.AluOpType.add, mybir.AluOpType.max)

            # ---- final layer + fused constraint epilogue ----
            # W6 streams in per-i4-block DMAs of [128, nwin*4*480] bf16;
            # outputs leave in per-(i4,bt) DMAs of [128, nwin*480] f32.
            def w6_block(i4):
                nwin = 4 if i4 < 6 else 1
                cols = nwin * 4 * WT
                t = wp.tile([128, 4 * 4 * WT], F8, tag="w6blk", name="w6blk", bufs=7)
                nc.sync.dma_start(t[:, 0:cols], w6_d[:, i4 * 4 * 4 * WT:i4 * 4 * 4 * WT + cols])
                return t

            hap = h5all[:]
            wblks = [w6_block(i4) for i4 in range(NI4)]
            for i4 in range(NI4):
                nwin = 4 if i4 < 6 else 1
                wblk = wblks[i4]
                for bt in range(NBT):
                    ob = op.tile([128, 4 * WT], BF16, tag="ob", name="ob", bufs=8)
                    wap = wblk[:]
                    pss = []
                    for w in range(nwin):
                        ps = pm.tile([128, 512], F32, tag="ps", name="ps")[:, 0:WT]
                        pss.append(ps)
                        for kk in range(2):
                            # DoubleRow fp8: two k-tiles per pass
                            lhsT = bass_rust.AP(
                                tensor=hap.tensor,
                                offset=hap.offset + (2 * kk) * BC + bt * 128,
                                ap=[[hap.ap[0][0], 128], [BC, 2], [1, 128]],
                            )
                            rhs = bass_rust.AP(
                                tensor=wap.tensor,
                                offset=wap.offset + (w * 4 + 2 * kk) * WT,
                                ap=[[wap.ap[0][0], 128], [WT, 2], [1, WT]],
                            )
                            nc.tensor.matmul(
                                ps[:], lhsT, rhs,
                                start=(kk == 0), stop=False,
                                perf_mode=mybir.MatmulPerfMode.DoubleRow,
                            )
                    for w in range(nwin):
                        p0 = 32 * w
                        nc.tensor.matmul(
                            pss[w][:],
                            xre_b[p0:p0 + 32, i4 * 512 + bt * 128:i4 * 512 + (bt + 1) * 128],
                            cg[p0:p0 + 32, i4 * WT:(i4 + 1) * WT],
                            start=False, stop=True, tile_position=(p0, 0),
                        )
                    for w in range(nwin):
                        if (w + bt) % 2 == 0:
                            nc.vector.tensor_copy(ob[:, w * WT:(w + 1) * WT], pss[w][:])
                        else:
                            nc.scalar.copy(ob[:, w * WT:(w + 1) * WT], pss[w][:])
                    nc.sync.dma_start(
                        y_d[bt * 128:(bt + 1) * 128, i4 * 4 * WT:i4 * 4 * WT + nwin * WT],
                        ob[:, 0:nwin * WT],
                    )


    nc.compile()
    return nc


def _host_prep(inputs):
    """Build per-core in_maps from the full inputs."""
    x_full = np.asarray(inputs["low_res_data"], np.float32).reshape(B, D_IN)
    labels = np.asarray(inputs["labels"]).astype(np.float32)
    W1 = np.asarray(inputs["W1"], np.float32)
    W6 = np.asarray(inputs["W6"], np.float32)
    b6 = np.asarray(inputs["b6"], np.float32)

    # per-timestep blend coefficients (match the reference formulas)
    t = np.arange(HIGH_T)
    seg = np.clip(t // UP, 0, LOW_T - 2)
    alpha = ((t - seg * UP) / UP).astype(np.float64)
    is_anchor = (t % UP) == 0
    interior = t < (LOW_T - 1) * UP
    blendf = np.where(is_anchor, 1.0, np.where(interior, 0.8, 0.0))
    c_d = np.where(is_anchor, 0.0, np.where(interior, 0.2, 1.0))
    c_start = blendf * (1.0 - alpha)
    c_end = blendf * alpha

    # G matrix, window-blocked: [128, NI4*480]; window i lives at partition
    # offset 32*(i%4), col block i//4.  Rows r=0..29 <-> x col 24*i + r,
    # row 30 = bias row (paired with the constant-1.0 row of xre).
    gmat = np.zeros((128, NI4 * WT), np.float64)
    for tt in range(HIGH_T):
        i, dt = divmod(tt, 80)
        i4, wpos = divmod(i, 4)
        p0 = 32 * wpos
        sl = seg[tt] - 4 * i
        for f in range(FEAT):
            col = i4 * WT + FEAT * dt + f
            gmat[p0 + FEAT * sl + f, col] += c_start[tt]
            gmat[p0 + FEAT * (sl + 1) + f, col] += c_end[tt]
            gmat[p0 + 30, col] = c_d[tt] * np.float64(b6[FEAT * tt + f])
    gmat = gmat.astype(np.float32).astype(BF16_NP)

    c_d_full = np.repeat(c_d, FEAT).astype(np.float32)
    w6p = (W6 * c_d_full[None, :]).astype(F8_NP)     # [512, 12000]
    # repack: w6r[p, ((i*4 + k)*480 + c)] = w6p[k*128 + p, i*480 + c]
    w6r = np.ascontiguousarray(
        w6p.reshape(4, 128, NW, WT).transpose(1, 2, 0, 3).reshape(128, NW * 4 * WT)
    )

    # W1 rearranged to the window-blocked xre layout (rows 30/31 zero)
    w1re = np.zeros((128, NI4 * 512), np.float32)
    for c in range(D_IN):
        i, r = divmod(c, 24)
        i4, wpos = divmod(i, 4)
        w1re[32 * wpos + r, i4 * 512:(i4 + 1) * 512] = W1[c, :]
    # wenc pack [128, 6144] bf16
    wenc = np.zeros((128, WENC_COLS), np.float32)
    wenc[:, OFF_W1:OFF_W1 + NI4 * 512] = w1re
    W2 = np.asarray(inputs["W2"], np.float32)
    for k in range(4):
        wenc[:, OFF_W2 + k * 256:OFF_W2 + (k + 1) * 256] = W2[k * 128:(k + 1) * 128, :]
    W3 = np.asarray(inputs["W3"], np.float32)
    for k in range(2):
        wenc[:, OFF_W3 + k * 128:OFF_W3 + (k + 1) * 128] = W3[k * 128:(k + 1) * 128, :]
    W4 = np.asarray(inputs["W4"], np.float32)
    wenc[:, OFF_W4A:OFF_W4A + 256] = W4[:128]
    wenc[0:16, OFF_W4B:OFF_W4B + 256] = W4[128:144]
    W5 = np.asarray(inputs["W5"], np.float32)
    for k in range(2):
        wenc[:, OFF_W5 + k * 512:OFF_W5 + (k + 1) * 512] = W5[k * 128:(k + 1) * 128, :]
    wenc = wenc.astype(F8_NP)

    # bias pack [128, 13] f32
    bias = np.zeros((128, 13), np.float32)
    b = {k: np.asarray(inputs[k], np.float32) for k in ["b1", "b2", "b3", "b4", "b5"]}
    for m in range(4):
        bias[:, OFF_B1 + m] = b["b1"][m * 128:(m + 1) * 128]
        bias[:, OFF_B5 + m] = b["b5"][m * 128:(m + 1) * 128]
    for m in range(2):
        bias[:, OFF_B2 + m] = b["b2"][m * 128:(m + 1) * 128]
        bias[:, OFF_B4 + m] = b["b4"][m * 128:(m + 1) * 128]
    bias[:, OFF_B3] = b["b3"]

    const_map = {
        "wenc": wenc,
        "bias": bias,
        "w6r": w6r,
        "embT": np.asarray(inputs["emb"], np.float32).astype(BF16_NP),
        "iota10": np.arange(NUM_CLASSES, dtype=np.float32).reshape(NUM_CLASSES, 1),
        "gmat": gmat,
    }

    in_maps = []
    for c in range(NCORES):
        sl = slice(c * BC, (c + 1) * BC)
        xc = x_full[sl]                                   # [512, 600]
        # xre window-blocked transpose [128, NI4*512] bf16:
        # xre[32*w + r, i4*512 + bt*128 + j] = x[bt*128+j, 96*i4 + 24*w + r]
        # rows 30 = 1.0 (G bias row), 31 = 0.0; block 6 only has window 0.
        xre = np.zeros((128, NI4 * 512), np.float32)
        xb = xc.reshape(NBT, 128, D_IN)                   # [bt, j, c]
        for i4 in range(NI4):
            nwin = 4 if i4 < 6 else 1
            for w in range(nwin):
                c0 = 96 * i4 + 24 * w
                ncols = min(24 + 6, D_IN - c0) if i4 == 6 else 30
                # window rows r=0..29 come from x cols c0..c0+30 (next window
                # overlap); last window: cols 576..599 -> 24 rows, rest 0
                blk = xb[:, :, c0:c0 + ncols]             # [bt, j, r]
                xre[32 * w:32 * w + blk.shape[2], i4 * 512:(i4 + 1) * 512] = (
                    blk.transpose(2, 0, 1).reshape(blk.shape[2], BC)
                )
            xre[32 * np.arange(nwin) + 30, i4 * 512:(i4 + 1) * 512] = 1.0
        m = dict(const_map)
        m["xre"] = xre.astype(BF16_NP)
        m["xre8"] = xre.astype(F8_NP)
        m["labf"] = labels[sl].reshape(1, BC).astype(BF16_NP)
        in_maps.append(m)
    return in_maps


_NC_CACHE = None


def kernel(**inputs) -> np.ndarray:
    global _NC_CACHE
    if _NC_CACHE is None:
        _NC_CACHE = _build_nc()
    nc = _NC_CACHE
    in_maps = _host_prep(inputs)
    res = bass_utils.run_bass_kernel_spmd(nc, in_maps, core_ids=list(range(NCORES)))
    out = np.concatenate([res.results[c]["y"] for c in range(NCORES)], axis=0)
    return out.astype(np.float32).reshape(B, HIGH_T, FEAT)
